# revision 1
# baseline (speedup 1.0000x reference)
import functools

import jax
import jax.numpy as jnp
import numpy as np

# nn_CapLayer: grouped 1x1 conv + 3-iter dynamic routing (capsule layer).
# Data-parallel over batch: 256 batch elements sharded 32-per-core across
# 8 NeuronCores; conv weight w and bias b_conv replicated. Routing is
# batch-local so there is no cross-device communication.

NUM_SHARED = 32
IN_DIM = 8
NUM_OUT = 10
OUT_DIM = 16
ROUTE_NUM = 3
EPS = 1e-20

N_CORES = 8
BS = 256
H = 6


def _squash(s):
    n2 = jnp.sum(s * s, axis=2, keepdims=True)
    n = jnp.sqrt(n2)
    return s * (n2 / (1.0 + n2) / (n + EPS))


def _caps_shard(x, w, b_conv):
    # x: (bs_l, NUM_SHARED*IN_DIM, h, h) on one core
    bs, _, h, ww = x.shape
    I = NUM_SHARED * h * ww
    xg = x.reshape(bs, NUM_SHARED, IN_DIM, h * ww)
    pred = jnp.einsum('bsip,soi->bsop', xg, w)
    pred = pred + b_conv.reshape(1, NUM_SHARED, NUM_OUT * OUT_DIM, 1)
    pred = pred.reshape(bs, NUM_SHARED, NUM_OUT, OUT_DIM, h * ww)
    pred = jnp.transpose(pred, (0, 2, 3, 1, 4)).reshape(bs, NUM_OUT, OUT_DIM, I)

    b = jnp.zeros((bs, NUM_OUT, I), dtype=x.dtype)
    v = None
    for r in range(ROUTE_NUM):
        c = jax.nn.softmax(b, axis=2)
        s = jnp.einsum('bjdi,bji->bjd', pred, c)
        v = _squash(s)
        if r < ROUTE_NUM - 1:
            b = b + jnp.einsum('bjd,bjdi->bji', v, pred)
    return v


@functools.cache
def _pmapped():
    return jax.pmap(_caps_shard, axis_name='cores')


def kernel(x: np.ndarray, w: np.ndarray, b_conv: np.ndarray) -> np.ndarray:
    bs = x.shape[0]
    shard = bs // N_CORES
    xs = np.ascontiguousarray(x.reshape(N_CORES, shard, *x.shape[1:]))
    ws = np.broadcast_to(w, (N_CORES,) + w.shape)
    bs_ = np.broadcast_to(b_conv, (N_CORES,) + b_conv.shape)
    v = _pmapped()(xs, ws, bs_)
    v = np.asarray(v)
    return v.reshape(bs, NUM_OUT, OUT_DIM)


# revision 3
# speedup vs baseline: 498.8799x; 498.8799x over previous
import functools

import jax
import jax.numpy as jnp
import numpy as np

# nn_CapLayer: grouped 1x1 conv + 3-iter dynamic routing (capsule layer).
#
# Data-parallel over batch: 256 batch elements sharded 32-per-core across
# 8 NeuronCores; conv weight w and bias b_conv replicated. Routing is
# batch-local so there is no cross-device communication.
#
# The routing is computed in FACTORED form: the (bs, 10, 16, 1152) `pred`
# tensor (189 MB fp32 over the full batch) is never materialized. Every
# contraction against pred is pushed through its low-rank structure
# pred = W·xg + bias, so the kernel is bounded by reading x once
# (9.4 MB) plus small per-iteration tensors (c, logits: ~1.5 MB/core).
#
#   pred[b,(j,d),(s,p)] = sum_i' w[s,(j,d),i'] xg[b,s,i',p] + b_conv[s,(j,d)]
#
#   s-compute:  s[b,j,d] = sum_i pred·c
#             = sum_{s,i'} w[s,(j,d),i']·y[b,j,s,i'] + sum_s bias[s,(j,d)]·cs[b,j,s]
#     with y[b,j,s,i'] = sum_p xg[b,s,i',p]·c[b,j,s,p],  cs = sum_p c
#
#   b-update:   L[b,j,(s,p)] += sum_d v·pred
#             = sum_i' g[b,j,s,i']·xg[b,s,i',p] + h[b,j,s]
#     with g = sum_d v[b,j,d]·w[s,(j,d),i'],  h = sum_d v·bias

NUM_SHARED = 32
IN_DIM = 8
NUM_OUT = 10
OUT_DIM = 16
ROUTE_NUM = 3
EPS = 1e-20

N_CORES = 8
BS = 256
H = 6
P = H * H  # 36 spatial positions
I = NUM_SHARED * P  # 1152 input capsules


def _squash(s):
    # s: (bs, J, D)
    n2 = jnp.sum(s * s, axis=2, keepdims=True)
    n = jnp.sqrt(n2)
    return s * (n2 / (1.0 + n2) / (n + EPS))


def _caps_shard(x, w, b_conv):
    # x: (bs_l, NUM_SHARED*IN_DIM, h, h) on one core
    bs = x.shape[0]
    xg = x.reshape(bs, NUM_SHARED, IN_DIM, P)               # (b, s, i', p)
    Wr = w.reshape(NUM_SHARED, NUM_OUT, OUT_DIM, IN_DIM)    # (s, j, d, i')
    Br = b_conv.reshape(NUM_SHARED, NUM_OUT, OUT_DIM)       # (s, j, d)

    # r = 0: c is uniform (softmax of zeros) -> s0 = mean_i pred
    xs0 = jnp.sum(xg, axis=3)                               # (b, s, i')
    s0 = jnp.einsum('bsi,sjdi->bjd', xs0, Wr) + P * jnp.sum(Br, axis=0)
    s0 = s0 * (1.0 / I)
    v = _squash(s0)

    L = None  # routing logits, shape (b, j, s, p); None means all-zero
    for r in range(1, ROUTE_NUM):
        # b-update with v from iteration r-1
        g = jnp.einsum('bjd,sjdi->bjsi', v, Wr)             # (b, j, s, i')
        h = jnp.einsum('bjd,sjd->bjs', v, Br)               # (b, j, s)
        dL = jnp.einsum('bjsi,bsip->bjsp', g, xg) + h[..., None]
        L = dL if L is None else L + dL

        # s-compute with c = softmax(L) over i = (s, p)
        Lf = L.reshape(bs, NUM_OUT, I)
        m = jnp.max(Lf, axis=2, keepdims=True)
        e = jnp.exp(Lf - m)
        c = (e / jnp.sum(e, axis=2, keepdims=True)).reshape(bs, NUM_OUT, NUM_SHARED, P)
        y = jnp.einsum('bjsp,bsip->bjsi', c, xg)            # (b, j, s, i')
        cs = jnp.sum(c, axis=3)                             # (b, j, s)
        s_r = jnp.einsum('bjsi,sjdi->bjd', y, Wr) + jnp.einsum('bjs,sjd->bjd', cs, Br)
        v = _squash(s_r)
    return v


@functools.cache
def _pmapped(n_cores: int):
    return jax.pmap(_caps_shard, axis_name='cores', devices=jax.devices()[:n_cores])


def kernel(x: np.ndarray, w: np.ndarray, b_conv: np.ndarray) -> np.ndarray:
    bs = x.shape[0]
    n_cores = N_CORES
    n_dev = len(jax.devices())
    while n_cores > 1 and (n_cores > n_dev or bs % n_cores != 0):
        n_cores //= 2
    shard = bs // n_cores
    xs = np.ascontiguousarray(x.reshape(n_cores, shard, *x.shape[1:]))
    ws = np.ascontiguousarray(np.broadcast_to(w, (n_cores,) + w.shape))
    bs_ = np.ascontiguousarray(np.broadcast_to(b_conv, (n_cores,) + b_conv.shape))
    v = _pmapped(n_cores)(xs, ws, bs_)
    v = np.asarray(v)
    return v.reshape(bs, NUM_OUT, OUT_DIM)
